# revision 1
# baseline (speedup 1.0000x reference)
"""AcceptRejectPooling2D on 8 Trainium2 NeuronCores.

Reference semantics (per 2x2 window, stride 2, NHWC):
    r  = relu(x)
    s  = sum(r); ss = sum(r*r)
    out = ss / s   if s > 0 else 0

Sharding: pure data parallel over batch (64 -> 8 per core). Each core
processes x_local [8, 64, 64, 256] -> y_local [8, 32, 32, 256].

Layout per core: rows (b, h) of length W*C = 16384 floats. Output row
p = (b, ho) needs input rows 2p (even h) and 2p+1 (odd h). 256 output
rows = 2 partition groups of 128. Row chunks of F floats stream through
SBUF; within a chunk the w-pair reduction is a strided tensor_add.
"""

import sys

if "/opt/trn_rl_repo" not in sys.path:
    sys.path.insert(0, "/opt/trn_rl_repo")

import numpy as np

_B, _H, _W, _C = 8, 64, 64, 256  # per-core shard
_HO, _WO = _H // 2, _W // 2
_NP = 128                         # SBUF partitions
_F = 2048                         # floats per row chunk (8 w * 256 c)
_FO = _F // 2
_NG = (_B * _HO) // _NP           # partition groups (2)
_NK = (_W * _C) // _F             # chunks per row (8)
_EPS = 1e-30

_CACHE = {}


def _pin_act_table(bacc, mybir):
    """Route every activation to natural_log_exp_and_others (which holds
    Relu, Square, Ln AND Exp) so the kernel needs exactly one ACT
    function-table load. The compiler's per-instruction greedy set choice
    otherwise alternates sets (~2.7us reload each). Only the in-memory
    choice list is edited; set ids / loaded table bytes are unchanged.
    """
    if getattr(bacc, "_arp_act_pin", False):
        return
    AF = mybir.ActivationFunctionType
    pin = {AF.Relu, AF.Square, AF.Ln, AF.Exp}
    orig = bacc.get_activation_tables

    def pinned(arch):
        return {
            name: (fns if name == "natural_log_exp_and_others" else fns - pin)
            for name, fns in orig(arch).items()
        }

    bacc.get_activation_tables = pinned
    bacc._arp_act_pin = True


def _build_nc():
    import concourse.bacc as bacc
    import concourse.tile as tile
    from concourse import mybir

    _pin_act_table(bacc, mybir)
    nc = bacc.Bacc("TRN2", target_bir_lowering=False, debug=False, num_devices=8)
    f32 = mybir.dt.float32
    x = nc.dram_tensor("x", [_B, _H, _W, _C], f32, kind="ExternalInput")
    y = nc.dram_tensor("y", [_B, _HO, _WO, _C], f32, kind="ExternalOutput")

    # [256, 2, 16384]: xv[(b, ho), par, (w, c)] with par = h % 2
    xv = x.ap().rearrange("b (hh par) w c -> (b hh) par (w c)", par=2)
    # [256, 8192]
    yv = y.ap().rearrange("b i j c -> (b i) (j c)")

    relu = mybir.ActivationFunctionType.Relu
    square = mybir.ActivationFunctionType.Square
    ln_f = mybir.ActivationFunctionType.Ln
    exp_f = mybir.ActivationFunctionType.Exp
    add = mybir.AluOpType.add

    with tile.TileContext(nc) as tc:
        with (
            tc.tile_pool(name="io", bufs=4) as io,
            tc.tile_pool(name="tmp", bufs=2) as tmp,
            tc.tile_pool(name="rq", bufs=3) as rq,
        ):
            def emit(g, c0, F, act_recip=False):
                # One iteration covers both h-rows (E|O fused along free dim)
                # of 128 output rows x F floats of row.
                FO = F // 2
                p0, p1 = g * _NP, (g + 1) * _NP
                EO = io.tile([_NP, 2 * F], f32, tag="EO")
                eov = EO[:].rearrange("p (par f) -> p par f", par=2)
                nc.sync.dma_start(eov, xv[p0:p1, :, c0:c0 + F])

                R = rq.tile([_NP, 2 * F], f32, tag="RQ")
                Q = rq.tile([_NP, 2 * F], f32, tag="RQ")
                sEO = tmp.tile([_NP, F], f32, tag="sEO")
                ssEO = tmp.tile([_NP, F], f32, tag="ssEO")
                s = tmp.tile([_NP, FO], f32, tag="s")
                ss = tmp.tile([_NP, FO], f32, tag="ss")
                t = tmp.tile([_NP, FO], f32, tag="t")
                o = tmp.tile([_NP, FO], f32, tag="o")

                def pairs(tile_):
                    # [128, 2F] -> even/odd w views [128, 2, F//512, 256]
                    v = tile_[:].rearrange(
                        "p (h w par c) -> p h w par c", h=2, par=2, c=_C
                    )
                    return v[:, :, :, 0, :], v[:, :, :, 1, :]

                def halfpair(tile_):
                    return tile_[:].rearrange("p (h w c) -> p h w c", h=2, c=_C)

                # relu + square, each one ACT pass over the fused tile
                nc.scalar.activation(R[:], EO[:], relu)
                Re, Ro = pairs(R)
                # w-pair adds for both h-rows in one op: sEO = [sE | sO]
                nc.vector.tensor_add(halfpair(sEO), Re, Ro)
                # s = (sE + eps) + sO ; eps guards 1/0 for all-zero windows
                nc.vector.scalar_tensor_tensor(
                    s[:], sEO[:, :FO], _EPS, sEO[:, FO:], op0=add, op1=add
                )
                if act_recip:
                    # 1/s = exp(-ln(s)) on ACT (max rel err ~5e-5) to offload
                    # the DVE, which is the bottleneck engine.
                    lt = tmp.tile([_NP, FO], f32, tag="lt")
                    nc.scalar.activation(lt[:], s[:], ln_f)
                    nc.scalar.activation(t[:], lt[:], exp_f, scale=-1.0)
                else:
                    nc.vector.reciprocal_approx_fast(t[:], s[:])

                nc.scalar.activation(Q[:], R[:], square)
                Qe, Qo = pairs(Q)
                nc.vector.tensor_add(halfpair(ssEO), Qe, Qo)
                nc.vector.tensor_add(ss[:], ssEO[:, :FO], ssEO[:, FO:])
                # NOTE: gpsimd offload of DVE ops is ~40% WORSE overall: DVE
                # tensor_tensor uses the shared SBUF port as its 2nd read
                # port, so concurrent GpSimd ops halve DVE throughput.
                nc.vector.tensor_mul(o[:], ss[:], t[:])

                nc.sync.dma_start(yv[p0:p1, c0 // 2:c0 // 2 + FO], o[:])

            # Warm the ACT function-table (~2.7us load) and DVE custom-op
            # path on dummy data so they overlap the first input DMA instead
            # of delaying the first real relu.
            warm0 = tmp.tile([_NP, 8], f32, tag="warm0")
            warm1 = tmp.tile([_NP, 8], f32, tag="warm1")
            nc.vector.memset(warm0[:], 1.0)
            nc.scalar.activation(warm1[:], warm0[:], relu)
            nc.scalar.activation(warm1[:], warm0[:], square)
            nc.scalar.activation(warm1[:], warm0[:], ln_f)
            nc.scalar.activation(warm1[:], warm0[:], exp_f)
            nc.vector.reciprocal_approx_fast(warm1[:], warm0[:])

            row = _W * _C
            full_idx = 0
            for g in range(_NG):
                c = 0
                if g == 0:
                    # fine-grained warmup chunks so compute starts early
                    for fw in (512, 512, 1024):
                        emit(g, c, fw)
                        c += fw
                # fine-grained cooldown chunks on the last group shrink the
                # (last compute -> last store) tail
                tail = (1024, 512, 512) if g == _NG - 1 else ()
                stop = row - sum(tail)
                while c < stop:
                    # 7 of 16 full chunks divide on ACT (exp(-ln)) to balance
                    # the two engines
                    emit(g, c, _F, act_recip=(full_idx % 2 == 1 and full_idx < 14))
                    full_idx += 1
                    c += _F
                for fw in tail:
                    # cooldown divisions on ACT: they sit at the end of the
                    # DVE critical path while ACT has tail slack
                    emit(g, c, fw, act_recip=g == _NG - 1)
                    c += fw

    nc.compile()
    return nc


def _get_nc():
    if "nc" not in _CACHE:
        _CACHE["nc"] = _build_nc()
    return _CACHE["nc"]


def kernel(x: np.ndarray) -> np.ndarray:
    from concourse.bass_utils import run_bass_kernel_spmd

    nc = _get_nc()
    x = np.ascontiguousarray(np.asarray(x, dtype=np.float32))
    shards = np.split(x, 8, axis=0)
    in_maps = [{"x": s} for s in shards]
    res = run_bass_kernel_spmd(nc, in_maps, list(range(8)))
    return np.concatenate([res.results[i]["y"] for i in range(8)], axis=0)



# revision 2
# speedup vs baseline: 1.0561x; 1.0561x over previous
"""AcceptRejectPooling2D on 8 Trainium2 NeuronCores.

Reference semantics (per 2x2 window, stride 2, NHWC):
    r  = relu(x)
    s  = sum(r); ss = sum(r*r)
    out = ss / s   if s > 0 else 0

Sharding: pure data parallel over batch (64 -> 8 per core). Each core
processes x_local [8, 64, 64, 256] -> y_local [8, 32, 32, 256].

Layout per core: rows (b, h) of length W*C = 16384 floats. Output row
p = (b, ho) needs input rows 2p (even h) and 2p+1 (odd h). 256 output
rows = 2 partition groups of 128. Row chunks of F floats stream through
SBUF; within a chunk the w-pair reduction is a strided tensor_add.

bf16 intermediate pipeline (rel-err budget 2e-2 allows it): ACT casts
relu(x) f32->bf16; squares and the window adds run on bf16 operands so
the DVE's 2x packed mode applies. Only s (recip input), t and the final
output stay f32.
"""

import sys

if "/opt/trn_rl_repo" not in sys.path:
    sys.path.insert(0, "/opt/trn_rl_repo")

import numpy as np

_B, _H, _W, _C = 8, 64, 64, 256  # per-core shard
_HO, _WO = _H // 2, _W // 2
_NP = 128                         # SBUF partitions
_F = 2048                         # floats per row chunk (8 w * 256 c)
_FO = _F // 2
_NG = (_B * _HO) // _NP           # partition groups (2)
_NK = (_W * _C) // _F             # chunks per row (8)
_EPS = 1e-30

_CACHE = {}


def _pin_act_table(bacc, mybir):
    """Route every activation to natural_log_exp_and_others (which holds
    Relu, Square, Ln AND Exp) so the kernel needs exactly one ACT
    function-table load. The compiler's per-instruction greedy set choice
    otherwise alternates sets (~2.7us reload each). Only the in-memory
    choice list is edited; set ids / loaded table bytes are unchanged.
    """
    if getattr(bacc, "_arp_act_pin", False):
        return
    AF = mybir.ActivationFunctionType
    pin = {AF.Relu, AF.Square, AF.Ln, AF.Exp}
    orig = bacc.get_activation_tables

    def pinned(arch):
        return {
            name: (fns if name == "natural_log_exp_and_others" else fns - pin)
            for name, fns in orig(arch).items()
        }

    bacc.get_activation_tables = pinned
    bacc._arp_act_pin = True


def _build_nc():
    import concourse.bacc as bacc
    import concourse.tile as tile
    from concourse import mybir

    _pin_act_table(bacc, mybir)
    nc = bacc.Bacc("TRN2", target_bir_lowering=False, debug=False, num_devices=8)
    f32 = mybir.dt.float32
    bf16 = mybir.dt.bfloat16
    x = nc.dram_tensor("x", [_B, _H, _W, _C], f32, kind="ExternalInput")
    y = nc.dram_tensor("y", [_B, _HO, _WO, _C], f32, kind="ExternalOutput")

    # [256, 2, 16384]: xv[(b, ho), par, (w, c)] with par = h % 2
    xv = x.ap().rearrange("b (hh par) w c -> (b hh) par (w c)", par=2)
    # [256, 8192]
    yv = y.ap().rearrange("b i j c -> (b i) (j c)")

    relu = mybir.ActivationFunctionType.Relu
    square = mybir.ActivationFunctionType.Square
    add = mybir.AluOpType.add

    with tile.TileContext(nc) as tc:
        with (
            tc.tile_pool(name="io", bufs=3) as io,
            tc.tile_pool(name="rq", bufs=4) as rq,
            tc.tile_pool(name="tmp", bufs=3) as tmp,
        ):
            def emit(g, c0, F, act_square=False):
                # One iteration covers both h-rows (E|O fused along free dim)
                # of 128 output rows x F floats of row.
                FO = F // 2
                p0, p1 = g * _NP, (g + 1) * _NP
                EO = io.tile([_NP, 2 * F], f32, tag="EO")
                eov = EO[:].rearrange("p (par f) -> p par f", par=2)
                nc.sync.dma_start(eov, xv[p0:p1, :, c0:c0 + F])

                R = rq.tile([_NP, 2 * F], bf16, tag="RQ")
                Q = rq.tile([_NP, 2 * F], bf16, tag="RQ")
                sw = tmp.tile([_NP, F], bf16, tag="sw")
                ssw = tmp.tile([_NP, F], bf16, tag="ssw")
                s = tmp.tile([_NP, FO], f32, tag="s")
                ss = tmp.tile([_NP, FO], bf16, tag="ss")
                t = tmp.tile([_NP, FO], f32, tag="t")
                o = tmp.tile([_NP, FO], f32, tag="o")

                def pairs(tile_):
                    # [128, 2F] -> even/odd w views [128, 2, F//512, 256]
                    v = tile_[:].rearrange(
                        "p (h w par c) -> p h w par c", h=2, par=2, c=_C
                    )
                    return v[:, :, :, 0, :], v[:, :, :, 1, :]

                def halfpair(tile_):
                    return tile_[:].rearrange("p (h w c) -> p h w c", h=2, c=_C)

                # relu + downcast to bf16 in one ACT pass
                nc.scalar.activation(R[:], EO[:], relu)
                Re, Ro = pairs(R)
                # w-pair adds for both h-rows in one bf16 op: sw = [sE | sO]
                nc.vector.tensor_add(halfpair(sw), Re, Ro)
                # s = (sE + eps) + sO in f32; eps guards 1/0 for zero windows
                nc.vector.scalar_tensor_tensor(
                    s[:], sw[:, :FO], _EPS, sw[:, FO:], op0=add, op1=add
                )
                nc.vector.reciprocal_approx_fast(t[:], s[:])

                if act_square:
                    nc.scalar.activation(Q[:], R[:], square)
                else:
                    nc.vector.tensor_mul(Q[:], R[:], R[:])
                Qe, Qo = pairs(Q)
                nc.vector.tensor_add(halfpair(ssw), Qe, Qo)
                nc.vector.tensor_add(ss[:], ssw[:, :FO], ssw[:, FO:])
                nc.vector.tensor_mul(o[:], ss[:], t[:])

                nc.sync.dma_start(yv[p0:p1, c0 // 2:c0 // 2 + FO], o[:])

            # Warm the ACT function-table (~2.7us load) and DVE custom-op
            # path on dummy data so they overlap the first input DMA instead
            # of delaying the first real relu.
            warm0 = tmp.tile([_NP, 8], f32, tag="warm0")
            warm1 = tmp.tile([_NP, 8], f32, tag="warm1")
            warmb = tmp.tile([_NP, 8], bf16, tag="warmb")
            nc.vector.memset(warm0[:], 1.0)
            nc.scalar.activation(warmb[:], warm0[:], relu)
            nc.scalar.activation(warmb[:], warmb[:], square)
            nc.vector.reciprocal_approx_fast(warm1[:], warm0[:])

            row = _W * _C
            full_idx = 0
            for g in range(_NG):
                c = 0
                if g == 0:
                    # fine-grained warmup chunks so compute starts early
                    for fw in (512, 512, 1024):
                        emit(g, c, fw)
                        c += fw
                # fine-grained cooldown chunks on the last group shrink the
                # (last compute -> last store) tail
                tail = (1024, 512, 512) if g == _NG - 1 else ()
                stop = row - sum(tail)
                while c < stop:
                    # half the full chunks square on ACT to balance engines
                    emit(g, c, _F, act_square=(full_idx % 2 == 1))
                    full_idx += 1
                    c += _F
                for fw in tail:
                    # cooldown squares on ACT: the DVE tail (recip+mul) is
                    # the critical path at the end while ACT has slack
                    emit(g, c, fw, act_square=True)
                    c += fw

    nc.compile()
    return nc


def _get_nc():
    if "nc" not in _CACHE:
        _CACHE["nc"] = _build_nc()
    return _CACHE["nc"]


def kernel(x: np.ndarray) -> np.ndarray:
    from concourse.bass_utils import run_bass_kernel_spmd

    nc = _get_nc()
    x = np.ascontiguousarray(np.asarray(x, dtype=np.float32))
    shards = np.split(x, 8, axis=0)
    in_maps = [{"x": s} for s in shards]
    res = run_bass_kernel_spmd(nc, in_maps, list(range(8)))
    return np.concatenate([res.results[i]["y"] for i in range(8)], axis=0)


# revision 9
# speedup vs baseline: 1.1129x; 1.0537x over previous
"""AcceptRejectPooling2D on 8 Trainium2 NeuronCores.

Reference semantics (per 2x2 window, stride 2, NHWC):
    r  = relu(x)
    s  = sum(r); ss = sum(r*r)
    out = ss / s   if s > 0 else 0

Sharding: pure data parallel over batch (64 -> 8 per core). Each core
processes x_local [8, 64, 64, 256] -> y_local [8, 32, 32, 256].

Layout per core: rows (b, h) of length W*C = 16384 floats. Output row
p = (b, ho) needs input rows 2p (even h) and 2p+1 (odd h). 256 output
rows = 2 partition groups of 128. Row chunks of F floats stream through
SBUF; within a chunk the w-pair reduction is a strided tensor_add.

bf16 intermediate pipeline (rel-err budget 2e-2 allows it): ACT casts
relu(x) f32->bf16; squares and the w-pair adds run on bf16 operands so
the DVE's 2x packed mode applies. s and ss promote to f32 at their
h-combine; out = ss / s is a single DVE tensor_tensor divide.
"""

import sys

if "/opt/trn_rl_repo" not in sys.path:
    sys.path.insert(0, "/opt/trn_rl_repo")

import numpy as np

_B, _H, _W, _C = 8, 64, 64, 256  # per-core shard
_HO, _WO = _H // 2, _W // 2
_NP = 128                         # SBUF partitions
_F = 2048                         # floats per row chunk (8 w * 256 c)
_FO = _F // 2
_NG = (_B * _HO) // _NP           # partition groups (2)
_EPS = 1e-30

_CACHE = {}


def _pin_act_table(bacc, mybir):
    """Route every activation to natural_log_exp_and_others (which holds
    Relu, Square, Ln AND Exp) so the kernel needs exactly one ACT
    function-table load. The compiler's per-instruction greedy set choice
    otherwise alternates sets (~2.7us reload each). Only the in-memory
    choice list is edited; set ids / loaded table bytes are unchanged.
    """
    if getattr(bacc, "_arp_act_pin", False):
        return
    AF = mybir.ActivationFunctionType
    pin = {AF.Relu, AF.Square, AF.Ln, AF.Exp}
    orig = bacc.get_activation_tables

    def pinned(arch):
        return {
            name: (fns if name == "natural_log_exp_and_others" else fns - pin)
            for name, fns in orig(arch).items()
        }

    bacc.get_activation_tables = pinned
    bacc._arp_act_pin = True


def _build_nc():
    import concourse.bacc as bacc
    import concourse.tile as tile
    from concourse import mybir

    _pin_act_table(bacc, mybir)
    nc = bacc.Bacc("TRN2", target_bir_lowering=False, debug=False, num_devices=8)
    f32 = mybir.dt.float32
    bf16 = mybir.dt.bfloat16
    x = nc.dram_tensor("x", [_B, _H, _W, _C], f32, kind="ExternalInput")
    y = nc.dram_tensor("y", [_B, _HO, _WO, _C], f32, kind="ExternalOutput")

    # [256, 2, 16384]: xv[(b, ho), par, (w, c)] with par = h % 2
    xv = x.ap().rearrange("b (hh par) w c -> (b hh) par (w c)", par=2)
    # [256, 8192]
    yv = y.ap().rearrange("b i j c -> (b i) (j c)")

    relu = mybir.ActivationFunctionType.Relu
    square = mybir.ActivationFunctionType.Square
    add = mybir.AluOpType.add

    with tile.TileContext(nc) as tc:
        with (
            tc.tile_pool(name="io", bufs=4) as io,
            tc.tile_pool(name="rq", bufs=4) as rq,
            tc.tile_pool(name="tmp", bufs=3) as tmp,
            tc.tile_pool(name="ot", bufs=3) as ot,
        ):
            def emit(g, c0, F, act_square, o_tile, o_off, o_w):
                # One iteration covers both h-rows (E|O fused along free dim)
                # of 128 output rows x F floats of row. Output lands in
                # o_tile[:, o_off:o_off+F/2]; caller stores when full.
                FO = F // 2
                p0, p1 = g * _NP, (g + 1) * _NP
                EO = io.tile([_NP, 2 * F], f32, tag="EO")
                eov = EO[:].rearrange("p (par f) -> p par f", par=2)
                nc.sync.dma_start(eov, xv[p0:p1, :, c0:c0 + F])

                R = rq.tile([_NP, 2 * F], bf16, tag="RQ")
                Q = rq.tile([_NP, 2 * F], bf16, tag="RQ")
                sw = tmp.tile([_NP, F], bf16, tag="sw")
                ssw = tmp.tile([_NP, F], bf16, tag="ssw")
                s = tmp.tile([_NP, FO], f32, tag="s")
                ss = tmp.tile([_NP, FO], f32, tag="ss")
                t = tmp.tile([_NP, FO], f32, tag="t")

                def pairs(tile_):
                    # [128, 2F] -> even/odd w views [128, 2, F//512, 256]
                    v = tile_[:].rearrange(
                        "p (h w par c) -> p h w par c", h=2, par=2, c=_C
                    )
                    return v[:, :, :, 0, :], v[:, :, :, 1, :]

                def halfpair(tile_):
                    return tile_[:].rearrange("p (h w c) -> p h w c", h=2, c=_C)

                # relu + downcast to bf16 in one ACT pass
                nc.scalar.activation(R[:], EO[:], relu)
                Re, Ro = pairs(R)
                # w-pair adds for both h-rows in one bf16 op: sw = [sE | sO]
                nc.vector.tensor_add(halfpair(sw), Re, Ro)
                # s = (sE + eps) + sO in f32; eps guards 0/0 zero windows
                nc.vector.scalar_tensor_tensor(
                    s[:], sw[:, :FO], _EPS, sw[:, FO:], op0=add, op1=add
                )
                nc.vector.reciprocal_approx_fast(t[:], s[:])

                if act_square:
                    nc.scalar.activation(Q[:], R[:], square)
                else:
                    nc.vector.tensor_mul(Q[:], R[:], R[:])
                Qe, Qo = pairs(Q)
                nc.vector.tensor_add(halfpair(ssw), Qe, Qo)
                nc.vector.tensor_add(ss[:], ssw[:, :FO], ssw[:, FO:])
                nc.vector.tensor_mul(o_tile[:, o_off:o_off + FO], ss[:], t[:])
                if o_off + FO == o_w:
                    nc.sync.dma_start(
                        yv[p0:p1, (c0 + F) // 2 - o_w:(c0 + F) // 2],
                        o_tile[:, :o_w],
                    )

            # Warm the ACT function-table (~2.7us load) and the DVE custom
            # recip path on dummy data so they overlap the first input DMA
            # instead of delaying the first real relu.
            warm0 = tmp.tile([_NP, 8], f32, tag="warm0")
            warm1 = tmp.tile([_NP, 8], f32, tag="warm1")
            warmb = tmp.tile([_NP, 8], bf16, tag="warmb")
            nc.vector.memset(warm0[:], 1.0)
            nc.scalar.activation(warmb[:], warm0[:], relu)
            nc.scalar.activation(warmb[:], warmb[:], square)
            nc.vector.reciprocal_approx_fast(warm1[:], warm0[:])

            row = _W * _C
            full_idx = 0
            for g in range(_NG):
                c = 0
                if g == 0:
                    # fine-grained warmup chunks so compute starts early;
                    # each stores its own (small) output immediately
                    for fw in (512, 512, 1024):
                        o_t = ot.tile([_NP, fw // 2], f32, tag="o")
                        emit(g, c, fw, False, o_t, 0, fw // 2)
                        c += fw
                # fine-grained cooldown chunks on the last group shrink the
                # (last compute -> last store) tail
                tail = (1024, 512, 512) if g == _NG - 1 else ()
                stop = row - sum(tail)
                # full chunks: pair outputs into 1 MiB stores; squares go to
                # ACT on 7 of 8 chunks to balance ACT vs DVE (DVE keeps the
                # recip+mul division tail)
                pend, pend_off = None, 0
                while c < stop:
                    if pend is None:
                        nfull = (stop - c) // _F
                        o_w = _FO * (2 if nfull >= 2 else 1)
                        pend = ot.tile([_NP, o_w], f32, tag="o")
                        pend_off, pend_w = 0, o_w
                    emit(g, c, _F, full_idx % 8 != 0, pend, pend_off, pend_w)
                    pend_off += _FO
                    if pend_off == pend_w:
                        pend = None
                    full_idx += 1
                    c += _F
                for fw in tail:
                    # cooldown squares on ACT: the DVE tail (divide) is the
                    # critical path at the end while ACT has slack
                    o_t = ot.tile([_NP, fw // 2], f32, tag="o")
                    emit(g, c, fw, True, o_t, 0, fw // 2)
                    c += fw

    nc.compile()
    return nc


def _get_nc():
    if "nc" not in _CACHE:
        _CACHE["nc"] = _build_nc()
    return _CACHE["nc"]


def kernel(x: np.ndarray) -> np.ndarray:
    from concourse.bass_utils import run_bass_kernel_spmd

    nc = _get_nc()
    x = np.ascontiguousarray(np.asarray(x, dtype=np.float32))
    shards = np.split(x, 8, axis=0)
    in_maps = [{"x": s} for s in shards]
    res = run_bass_kernel_spmd(nc, in_maps, list(range(8)))
    return np.concatenate([res.results[i]["y"] for i in range(8)], axis=0)
